# revision 16
# baseline (speedup 1.0000x reference)
"""GNN message-passing block on 8 Trainium2 NeuronCores.

Full (unsharded) numpy inputs in, full output out.

Sharding: batch dim across core groups (B=2 -> 4 cores per batch); within a
batch, edges are partitioned by receiver quarter (the scatter_add target
range), so each core owns a disjoint receiver range and no cross-core
communication is needed. Host-side shard construction sorts each core's
edges by receiver window (128 receivers), pads each window to a multiple of
128 edges, and lays out the sender node features in schedule order
(feature-major bf16) so the device streams them sequentially as matmul lhsT.

Per-core device program (bf16 matmuls, f32 PSUM):
  phase 0.5: y[n,:] = nodes_q[n,:] @ W_msg[128:,:] + b_msg  (col 128 = -mean)
  per 128-edge tile:
    psum[e,0:132] = S_T.T @ W1s_aug + SelT.T @ y_win   (col 128 = -mean(msg))
      Sel/SelT are one-hot receiver matrices built on DVE via is_equal
      against iota constants (SelT input row replicated via DMA broadcast).
    LN: ACT square+accum -> var; ACT sqrt; DVE reciprocal; normalize -> bf16
    scatter: psum_w[f,0:128] += msgs.T @ Sel   (segment-sum by receiver)
  per window: flush psum_w -> inbox (bf16, feature-major [128f, NQ])
  phase 2: out_pre = nodes_q@Wn_top + inbox@(g1*Wn_bot) + deg*(be1@Wn_bot)
           + b_node (rank-2 matmul), then LN2 with g2/be2 -> f32 out.
"""

import os
import numpy as np
import ml_dtypes

BF16 = ml_dtypes.bfloat16
P = 128
NC = 132          # matmul N: 128 features + col 128 = -mean + 3 pad
CH_TILES = 32     # tiles per sender-feature load chunk (32*128 edges = 1MB)
EPS = 1e-5
NCORES = 8

# set by test harness for profiling
_TRACE = False
LAST_EXEC_NS = None
LAST_RESULTS = None


# ----------------------------------------------------------------------------
# host-side schedule + per-core tensor prep
# ----------------------------------------------------------------------------

def _dims(nodes):
    B, N, D = nodes.shape
    assert D == P
    Q = NCORES // B
    NQR = -(-N // Q)              # receivers per quarter (real)
    NW = -(-NQR // P)             # windows per quarter
    NQ = NW * P
    return B, N, Q, NQR, NW, NQ


def _make_schedule(counts, NW):
    T = np.maximum(-(-counts.max(axis=0) // P), 1)   # tiles per window
    NT = int(T.sum())
    pad = (-NT) % 4
    if pad:
        T[NW - 1] += pad
        NT += pad
    cell_off = np.zeros(NW + 1, np.int64)
    cell_off[1:] = np.cumsum(T)
    tiles = np.repeat(np.arange(NW), T)              # tile -> window
    return dict(T=T, NT=NT, cell_off=cell_off, tiles=tiles, NW=NW)


def _aug(Wcols):
    K = Wcols.shape[0]
    out = np.zeros((K, NC), np.float32)
    out[:, :P] = Wcols
    out[:, P] = -Wcols.mean(axis=1)
    return out


def _prep(nodes, senders, receivers, W_msg, b_msg, W_node, b_node,
          g1, be1, g2, be2):
    B, N, Q, NQR, NW, NQ = _dims(nodes)

    W1s = W_msg[:P, :].astype(np.float32)
    W1r = W_msg[P:, :].astype(np.float32)
    Wn_top = W_node[:P, :].astype(np.float32)
    Wn_bot = W_node[P:, :].astype(np.float32)
    WnbotF = (g1[:, None] * Wn_bot).astype(np.float32)
    v = (be1 @ Wn_bot).astype(np.float32)
    w1s_aug = _aug(W1s).astype(BF16)
    w1r_aug = _aug(W1r).astype(BF16)
    baug = np.zeros((1, NC), np.float32)
    baug[0, :P] = b_msg
    baug[0, P] = -b_msg.mean()
    baug = np.tile(baug, (P, 1)).astype(BF16)
    wntop_aug = _aug(Wn_top).astype(BF16)
    wnbot_aug = _aug(WnbotF).astype(BF16)
    vb2 = np.zeros((2, NC), np.float32)
    vb2[0, :P] = v
    vb2[0, P] = -v.mean()
    vb2[1, :P] = b_node
    vb2[1, P] = -b_node.mean()
    vb2 = vb2.astype(BF16)
    g2rep = np.tile(g2[None, :], (P, 1)).astype(np.float32)
    b2rep = np.tile(be2[None, :], (P, 1)).astype(np.float32)
    iotapw = np.tile(np.arange(P, dtype=np.float32)[:, None], (1, 512)).astype(BF16)
    iotaf4 = np.tile(np.tile(np.arange(P, dtype=np.float32)[None, :], (1, 4)),
                     (P, 1)).astype(BF16)

    core_edges = []
    counts = np.zeros((NCORES, NW), np.int64)
    for c in range(NCORES):
        b, q = c // Q, c % Q
        r0 = q * NQR
        r1 = min(r0 + NQR, N)
        m = (receivers[b] >= r0) & (receivers[b] < r1)
        s = senders[b][m].astype(np.int64)
        r = (receivers[b][m] - r0).astype(np.int64)
        w = r >> 7
        counts[c] = np.bincount(w, minlength=NW)
        core_edges.append((b, q, s, r, w))
    sched = _make_schedule(counts, NW)
    NT = sched["NT"]
    cell_off = sched["cell_off"]

    in_maps = []
    nodes_bf_cache = {}
    for c in range(NCORES):
        b, q, s, r, w = core_edges[c]
        if b not in nodes_bf_cache:
            nodes_bf_cache[b] = nodes[b].astype(BF16)
        order = np.argsort(w, kind="stable")
        ws = w[order]
        starts = np.searchsorted(ws, np.arange(NW))
        ranks = np.arange(len(order)) - starts[ws]
        slots = cell_off[ws] * P + ranks
        rv_arr = np.full(NT * P, 200.0, np.float32)
        rv_arr[slots] = (r[order] & 127).astype(np.float32)
        sgathT = np.zeros((P, NT * P), BF16)
        sgathT[:, slots] = nodes_bf_cache[b][s[order]].T
        rvp = np.ascontiguousarray(rv_arr.reshape(NT, P).T).astype(BF16)
        rvf = np.ascontiguousarray(rv_arr.reshape(NT // 4, 512)).astype(BF16)
        r0 = q * NQR
        nqr_c = min(NQR, N - r0)
        nqT = np.zeros((P, NQ), BF16)
        nqT[:, :nqr_c] = nodes[b, r0:r0 + nqr_c, :].T.astype(BF16)
        deg = np.bincount(r, minlength=NQ).astype(np.float32)
        degones = np.stack([deg, np.ones(NQ, np.float32)]).astype(BF16)
        in_maps.append({
            "sgathT": sgathT,
            "nqT": nqT,
            "degones": degones,
            "rvp": rvp,
            "rvf": rvf,
            "w1s": w1s_aug, "w1r": w1r_aug, "baug": baug,
            "wntop": wntop_aug, "wnbot": wnbot_aug, "vb2": vb2,
            "g2rep": g2rep, "b2rep": b2rep, "iotapw": iotapw, "iotaf4": iotaf4,
        })
    meta = dict(B=B, N=N, Q=Q, NQR=NQR, NW=NW, NQ=NQ)
    return sched, in_maps, meta


# ----------------------------------------------------------------------------
# device program
# ----------------------------------------------------------------------------

def _build(sched, meta):
    import concourse.bacc as bacc
    import concourse.tile as tile
    from concourse import mybir
    from contextlib import ExitStack

    dt = mybir.dt
    AF = mybir.ActivationFunctionType
    OP = mybir.AluOpType

    NW, NQ = meta["NW"], meta["NQ"]
    NT = sched["NT"]
    tiles = sched["tiles"]
    cell_off = sched["cell_off"]

    nc = bacc.Bacc("TRN2", target_bir_lowering=False, debug=False,
                   enable_asserts=True, num_devices=NCORES)

    def din(name, shape, dd):
        return nc.dram_tensor(name, shape, dd, kind="ExternalInput").ap()

    sgathT = din("sgathT", [P, NT * P], dt.bfloat16)
    nqT = din("nqT", [P, NQ], dt.bfloat16)
    degones = din("degones", [2, NQ], dt.bfloat16)
    rvp = din("rvp", [P, NT], dt.bfloat16)
    rvf = din("rvf", [NT // 4, 512], dt.bfloat16)
    w1s = din("w1s", [P, NC], dt.bfloat16)
    w1r = din("w1r", [P, NC], dt.bfloat16)
    baug = din("baug", [P, NC], dt.bfloat16)
    wntop = din("wntop", [P, NC], dt.bfloat16)
    wnbot = din("wnbot", [P, NC], dt.bfloat16)
    vb2 = din("vb2", [2, NC], dt.bfloat16)
    g2rep = din("g2rep", [P, P], dt.float32)
    b2rep = din("b2rep", [P, P], dt.float32)
    iotapw = din("iotapw", [P, 512], dt.bfloat16)
    iotaf4 = din("iotaf4", [P, 512], dt.bfloat16)
    outp = nc.dram_tensor("out", [NQ, P], dt.float32, kind="ExternalOutput").ap()

    with tile.TileContext(nc) as tc, ExitStack() as ctx:
        big = ctx.enter_context(tc.tile_pool(name="big", bufs=1))
        gpool = ctx.enter_context(tc.tile_pool(name="g", bufs=3))
        rvpool = ctx.enter_context(tc.tile_pool(name="rvbc", bufs=3))
        selpool = ctx.enter_context(tc.tile_pool(name="sel", bufs=3))
        mpool = ctx.enter_context(tc.tile_pool(name="msgs", bufs=3))
        sqpool = ctx.enter_context(tc.tile_pool(name="sq", bufs=2))
        smpool = ctx.enter_context(tc.tile_pool(name="sm", bufs=6))
        outpool = ctx.enter_context(tc.tile_pool(name="outs", bufs=3))
        pm = ctx.enter_context(tc.tile_pool(name="pm", bufs=3, space="PSUM"))
        pw = ctx.enter_context(tc.tile_pool(name="pw", bufs=2, space="PSUM"))
        p2 = ctx.enter_context(tc.tile_pool(name="p2", bufs=2, space="PSUM"))

        def load(name, src, shape, dd):
            t = big.tile(shape, dd, tag=name)
            nc.sync.dma_start(t[:], src[:])
            return t

        rvp_sb = load("rvp", rvp, [P, NT], dt.bfloat16)
        nqT_sb = load("nqT", nqT, [P, NQ], dt.bfloat16)
        degones_sb = load("degones", degones, [2, NQ], dt.bfloat16)
        w1s_sb = load("w1s", w1s, [P, NC], dt.bfloat16)
        w1r_sb = load("w1r", w1r, [P, NC], dt.bfloat16)
        baug_sb = load("baug", baug, [P, NC], dt.bfloat16)
        wntop_sb = load("wntop", wntop, [P, NC], dt.bfloat16)
        wnbot_sb = load("wnbot", wnbot, [P, NC], dt.bfloat16)
        vb2_sb = load("vb2", vb2, [2, NC], dt.bfloat16)
        g2rep_sb = load("g2rep", g2rep, [P, P], dt.float32)
        b2rep_sb = load("b2rep", b2rep, [P, P], dt.float32)
        iotapw_sb = load("iotapw", iotapw, [P, 512], dt.bfloat16)
        iotaf4_sb = load("iotaf4", iotaf4, [P, 512], dt.bfloat16)
        y_sb = big.tile([P, NW * NC], dt.bfloat16, tag="y")
        inbox = big.tile([P, NQ], dt.bfloat16, tag="inbox")
        eps_sb = big.tile([P, 1], dt.float32, tag="eps")
        nc.vector.memset(eps_sb[:], float(EPS))

        # ---- phase 0.5: y = nodes_q @ W1r_aug + b_aug
        for w in range(NW):
            yp = p2.tile([P, NC], dt.float32, tag="p2")
            nc.tensor.matmul(out=yp[:], lhsT=nqT_sb[:, w * P:(w + 1) * P],
                             rhs=w1r_sb[:], start=True, stop=True)
            nc.vector.tensor_tensor(
                out=y_sb[:, w * NC:(w + 1) * NC], in0=yp[:],
                in1=baug_sb[:], op=OP.add)

        # ---- main tile loop
        gt = None
        gt_base = 0
        psw = None
        sel4 = selT4 = None
        for t in range(NT):
            w = int(tiles[t])
            if t % CH_TILES == 0:
                ntile = min(CH_TILES, NT - t)
                gt = gpool.tile([P, CH_TILES * P], dt.bfloat16, tag="gt")
                nc.sync.dma_start(gt[:, 0:ntile * P],
                                  sgathT[:, t * P:(t + ntile) * P])
                gt_base = t
            if t % 4 == 0:
                g4 = t // 4
                rvbc = rvpool.tile([P, 512], dt.bfloat16, tag="rvbc")
                nc.sync.dma_start(rvbc[:], rvf[g4:g4 + 1, :].to_broadcast((P, 512)))
                sel4 = selpool.tile([P, 4, P], dt.bfloat16, tag="sel4")
                nc.vector.tensor_tensor(
                    out=sel4[:],
                    in0=rvp_sb[:, t:t + 4].to_broadcast([P, 4, P]),
                    in1=iotaf4_sb[:].rearrange("p (a b) -> p a b", b=P),
                    op=OP.is_equal)
                selT4 = selpool.tile([P, 4, P], dt.bfloat16, tag="selT4")
                nc.vector.tensor_tensor(
                    out=selT4[:],
                    in0=iotapw_sb[:].rearrange("p (a b) -> p a b", b=P),
                    in1=rvbc[:].rearrange("p (a b) -> p a b", b=P),
                    op=OP.is_equal)

            toff = t - gt_base
            psm = pm.tile([P, NC], dt.float32, tag="pm")
            nc.tensor.matmul(out=psm[:], lhsT=gt[:, toff * P:(toff + 1) * P],
                             rhs=w1s_sb[:], start=True, stop=False)
            nc.tensor.matmul(out=psm[:], lhsT=selT4[:, t % 4, :],
                             rhs=y_sb[:, w * NC:(w + 1) * NC],
                             start=False, stop=True)
            negmu = smpool.tile([P, 1], dt.float32, tag="negmu")
            nc.vector.tensor_copy(out=negmu[:], in_=psm[:, P:P + 1])
            sq = sqpool.tile([P, P], dt.bfloat16, tag="sq")
            ssq = smpool.tile([P, 1], dt.float32, tag="ssq")
            nc.scalar.activation(sq[:], psm[:, 0:P], AF.Square,
                                 bias=negmu[:], scale=1.0, accum_out=ssq[:])
            std = smpool.tile([P, 1], dt.float32, tag="std")
            nc.scalar.activation(std[:], ssq[:], AF.Sqrt,
                                 bias=eps_sb[:], scale=1.0 / P)
            rstd = smpool.tile([P, 1], dt.float32, tag="rstd")
            nc.vector.reciprocal(rstd[:], std[:])
            msgs = mpool.tile([P, P], dt.bfloat16, tag="msgs")
            if t % 2 == 0:
                nc.vector.tensor_scalar(
                    out=msgs[:], in0=psm[:, 0:P], scalar1=negmu[:],
                    scalar2=rstd[:], op0=OP.add, op1=OP.mult)
            else:
                nmr = smpool.tile([P, 1], dt.float32, tag="nmr")
                nc.vector.tensor_mul(out=nmr[:], in0=negmu[:], in1=rstd[:])
                nc.scalar.activation(msgs[:], psm[:, 0:P], AF.Identity,
                                     bias=nmr[:], scale=rstd[:])
            first = t == cell_off[w]
            last = t == cell_off[w + 1] - 1
            if first:
                psw = pw.tile([P, P], dt.float32, tag="pw")
            nc.tensor.matmul(out=psw[:], lhsT=msgs[:], rhs=sel4[:, t % 4, :],
                             start=first, stop=last)
            if last:
                nc.vector.tensor_copy(out=inbox[:, w * P:(w + 1) * P], in_=psw[:])

        # ---- phase 2
        for w in range(NW):
            ps = p2.tile([P, NC], dt.float32, tag="p2")
            sl = slice(w * P, (w + 1) * P)
            nc.tensor.matmul(out=ps[:], lhsT=degones_sb[:, sl], rhs=vb2_sb[:],
                             start=True, stop=False)
            nc.tensor.matmul(out=ps[:], lhsT=nqT_sb[:, sl], rhs=wntop_sb[:],
                             start=False, stop=False)
            nc.tensor.matmul(out=ps[:], lhsT=inbox[:, sl], rhs=wnbot_sb[:],
                             start=False, stop=True)
            negmu2 = smpool.tile([P, 1], dt.float32, tag="negmu2")
            nc.vector.tensor_copy(out=negmu2[:], in_=ps[:, P:P + 1])
            sq2 = sqpool.tile([P, P], dt.bfloat16, tag="sq2")
            ssq2 = smpool.tile([P, 1], dt.float32, tag="ssq2")
            nc.scalar.activation(sq2[:], ps[:, 0:P], AF.Square,
                                 bias=negmu2[:], scale=1.0, accum_out=ssq2[:])
            std2 = smpool.tile([P, 1], dt.float32, tag="std2")
            nc.scalar.activation(std2[:], ssq2[:], AF.Sqrt,
                                 bias=eps_sb[:], scale=1.0 / P)
            rstd2 = smpool.tile([P, 1], dt.float32, tag="rstd2")
            nc.vector.reciprocal(rstd2[:], std2[:])
            tn = outpool.tile([P, P], dt.float32, tag="tn")
            nc.vector.tensor_scalar(out=tn[:], in0=ps[:, 0:P], scalar1=negmu2[:],
                                    scalar2=rstd2[:], op0=OP.add, op1=OP.mult)
            t2 = outpool.tile([P, P], dt.float32, tag="t2")
            nc.vector.tensor_tensor(out=t2[:], in0=tn[:], in1=g2rep_sb[:],
                                    op=OP.mult)
            osb = outpool.tile([P, P], dt.float32, tag="osb")
            nc.vector.tensor_tensor(out=osb[:], in0=t2[:], in1=b2rep_sb[:],
                                    op=OP.add)
            nc.sync.dma_start(outp[sl, :], osb[:])

    nc.compile()
    return nc


# ----------------------------------------------------------------------------
# entry point
# ----------------------------------------------------------------------------

def kernel(nodes, senders, receivers, W_msg, b_msg, W_node, b_node,
           g1, be1, g2, be2):
    global LAST_EXEC_NS, LAST_RESULTS
    from concourse.bass_utils import run_bass_kernel_spmd

    nodes = np.asarray(nodes, np.float32)
    sched, in_maps, meta = _prep(
        nodes, np.asarray(senders), np.asarray(receivers),
        np.asarray(W_msg, np.float32), np.asarray(b_msg, np.float32),
        np.asarray(W_node, np.float32), np.asarray(b_node, np.float32),
        np.asarray(g1, np.float32), np.asarray(be1, np.float32),
        np.asarray(g2, np.float32), np.asarray(be2, np.float32))
    nc = _build(sched, meta)
    res = run_bass_kernel_spmd(nc, in_maps, list(range(NCORES)), trace=_TRACE)
    LAST_EXEC_NS = res.exec_time_ns
    LAST_RESULTS = res
    B, N, Q, NQR = meta["B"], meta["N"], meta["Q"], meta["NQR"]
    out = np.zeros((B, N, P), np.float32)
    for c in range(NCORES):
        b, q = c // Q, c % Q
        r0 = q * NQR
        r1 = min(r0 + NQR, N)
        out[b, r0:r1, :] = res.results[c]["out"][:r1 - r0, :]
    return out


# revision 18
# speedup vs baseline: 1.2629x; 1.2629x over previous
"""GNN message-passing block on 8 Trainium2 NeuronCores.

Full (unsharded) numpy inputs in, full output out.

Sharding: batch dim across core groups (B=2 -> 4 cores per batch); within a
batch, edges are partitioned by receiver quarter (the scatter_add target
range), so each core owns a disjoint receiver range and no cross-core
communication is needed. Host-side shard construction sorts each core's
edges by receiver window (128 receivers), pads each window to a multiple of
128 edges, and lays out the sender node features in schedule order
(feature-major bf16) so the device streams them sequentially as matmul lhsT.

Per-core device program (bf16 matmuls, f32 PSUM):
  phase 0.5: y[n,:] = nodes_q[n,:] @ W_msg[128:,:] + b_msg  (col 128 = -mean)
  per 128-edge tile:
    psum[e,0:132] = S_T.T @ W1s_aug + SelT.T @ y_win   (col 128 = -mean(msg))
      Sel/SelT are one-hot receiver matrices built on DVE via is_equal
      against iota constants (SelT input row replicated via DMA broadcast).
    LN: ACT square+accum -> var; ACT sqrt; DVE reciprocal; normalize -> bf16
    scatter: psum_w[f,0:128] += msgs.T @ Sel   (segment-sum by receiver)
  per window: flush psum_w -> inbox (bf16, feature-major [128f, NQ])
  phase 2: out_pre = nodes_q@Wn_top + inbox@(g1*Wn_bot) + deg*(be1@Wn_bot)
           + b_node (rank-2 matmul), then LN2 with g2/be2 -> f32 out.
"""

import os
import numpy as np
import ml_dtypes

BF16 = ml_dtypes.bfloat16
P = 128
NC = 132          # matmul N: 128 features + col 128 = -mean + 3 pad
CH_TILES = 32     # tiles per sender-feature load chunk (32*128 edges = 1MB)
EPS = 1e-5
NCORES = 8

# set by test harness for profiling
_TRACE = False
LAST_EXEC_NS = None
LAST_RESULTS = None


# ----------------------------------------------------------------------------
# host-side schedule + per-core tensor prep
# ----------------------------------------------------------------------------

def _dims(nodes):
    B, N, D = nodes.shape
    assert D == P
    Q = NCORES // B
    NQR = -(-N // Q)              # receivers per quarter (real)
    NW = -(-NQR // P)             # windows per quarter
    NQ = NW * P
    return B, N, Q, NQR, NW, NQ


def _make_schedule(counts, NW):
    T = np.maximum(-(-counts.max(axis=0) // P), 1)   # tiles per window
    NT = int(T.sum())
    pad = (-NT) % 4
    if pad:
        T[NW - 1] += pad
        NT += pad
    cell_off = np.zeros(NW + 1, np.int64)
    cell_off[1:] = np.cumsum(T)
    tiles = np.repeat(np.arange(NW), T)              # tile -> window
    return dict(T=T, NT=NT, cell_off=cell_off, tiles=tiles, NW=NW)


def _aug(Wcols):
    K = Wcols.shape[0]
    out = np.zeros((K, NC), np.float32)
    out[:, :P] = Wcols
    out[:, P] = -Wcols.mean(axis=1)
    return out


def _prep(nodes, senders, receivers, W_msg, b_msg, W_node, b_node,
          g1, be1, g2, be2):
    B, N, Q, NQR, NW, NQ = _dims(nodes)

    W1s = W_msg[:P, :].astype(np.float32)
    W1r = W_msg[P:, :].astype(np.float32)
    Wn_top = W_node[:P, :].astype(np.float32)
    Wn_bot = W_node[P:, :].astype(np.float32)
    WnbotF = (g1[:, None] * Wn_bot).astype(np.float32)
    v = (be1 @ Wn_bot).astype(np.float32)
    w1s_aug = _aug(W1s).astype(BF16)
    w1r_aug = _aug(W1r).astype(BF16)
    baug = np.zeros((1, NC), np.float32)
    baug[0, :P] = b_msg
    baug[0, P] = -b_msg.mean()
    baug = np.tile(baug, (P, 1)).astype(BF16)
    wntop_aug = _aug(Wn_top).astype(BF16)
    wnbot_aug = _aug(WnbotF).astype(BF16)
    vb2 = np.zeros((2, NC), np.float32)
    vb2[0, :P] = v
    vb2[0, P] = -v.mean()
    vb2[1, :P] = b_node
    vb2[1, P] = -b_node.mean()
    vb2 = vb2.astype(BF16)
    g2rep = np.tile(g2[None, :], (P, 1)).astype(np.float32)
    b2rep = np.tile(be2[None, :], (P, 1)).astype(np.float32)
    iotapw = np.tile(np.arange(P, dtype=np.float32)[:, None], (1, 512)).astype(BF16)
    iotaf4 = np.tile(np.tile(np.arange(P, dtype=np.float32)[None, :], (1, 4)),
                     (P, 1)).astype(BF16)

    core_edges = []
    counts = np.zeros((NCORES, NW), np.int64)
    for c in range(NCORES):
        b, q = c // Q, c % Q
        r0 = q * NQR
        r1 = min(r0 + NQR, N)
        m = (receivers[b] >= r0) & (receivers[b] < r1)
        s = senders[b][m].astype(np.int64)
        r = (receivers[b][m] - r0).astype(np.int64)
        w = r >> 7
        counts[c] = np.bincount(w, minlength=NW)
        core_edges.append((b, q, s, r, w))
    sched = _make_schedule(counts, NW)
    NT = sched["NT"]
    cell_off = sched["cell_off"]

    in_maps = []
    nodes_bf_cache = {}
    for c in range(NCORES):
        b, q, s, r, w = core_edges[c]
        if b not in nodes_bf_cache:
            nodes_bf_cache[b] = nodes[b].astype(BF16)
        order = np.argsort(w, kind="stable")
        ws = w[order]
        starts = np.searchsorted(ws, np.arange(NW))
        ranks = np.arange(len(order)) - starts[ws]
        slots = cell_off[ws] * P + ranks
        rv_arr = np.full(NT * P, 200.0, np.float32)
        rv_arr[slots] = (r[order] & 127).astype(np.float32)
        sgathT = np.zeros((P, NT * P), BF16)
        sgathT[:, slots] = nodes_bf_cache[b][s[order]].T
        rvp = np.ascontiguousarray(rv_arr.reshape(NT, P).T).astype(BF16)
        rvf = np.ascontiguousarray(rv_arr.reshape(NT // 4, 512)).astype(BF16)
        r0 = q * NQR
        nqr_c = min(NQR, N - r0)
        nqT = np.zeros((P, NQ), BF16)
        nqT[:, :nqr_c] = nodes[b, r0:r0 + nqr_c, :].T.astype(BF16)
        deg = np.bincount(r, minlength=NQ).astype(np.float32)
        degones = np.stack([deg, np.ones(NQ, np.float32)]).astype(BF16)
        in_maps.append({
            "sgathT": sgathT,
            "nqT": nqT,
            "degones": degones,
            "rvp": rvp,
            "rvf": rvf,
            "w1s": w1s_aug, "w1r": w1r_aug, "baug": baug,
            "wntop": wntop_aug, "wnbot": wnbot_aug, "vb2": vb2,
            "g2rep": g2rep, "b2rep": b2rep, "iotapw": iotapw, "iotaf4": iotaf4,
        })
    meta = dict(B=B, N=N, Q=Q, NQR=NQR, NW=NW, NQ=NQ)
    return sched, in_maps, meta


# ----------------------------------------------------------------------------
# device program
# ----------------------------------------------------------------------------

def _build(sched, meta):
    import concourse.bacc as bacc
    import concourse.tile as tile
    from concourse import mybir
    from contextlib import ExitStack

    dt = mybir.dt
    AF = mybir.ActivationFunctionType
    OP = mybir.AluOpType

    NW, NQ = meta["NW"], meta["NQ"]
    NT = sched["NT"]
    tiles = sched["tiles"]
    cell_off = sched["cell_off"]

    nc = bacc.Bacc("TRN2", target_bir_lowering=False, debug=False,
                   enable_asserts=True, num_devices=NCORES)

    def din(name, shape, dd):
        return nc.dram_tensor(name, shape, dd, kind="ExternalInput").ap()

    sgathT = din("sgathT", [P, NT * P], dt.bfloat16)
    nqT = din("nqT", [P, NQ], dt.bfloat16)
    degones = din("degones", [2, NQ], dt.bfloat16)
    rvp = din("rvp", [P, NT], dt.bfloat16)
    rvf = din("rvf", [NT // 4, 512], dt.bfloat16)
    w1s = din("w1s", [P, NC], dt.bfloat16)
    w1r = din("w1r", [P, NC], dt.bfloat16)
    baug = din("baug", [P, NC], dt.bfloat16)
    wntop = din("wntop", [P, NC], dt.bfloat16)
    wnbot = din("wnbot", [P, NC], dt.bfloat16)
    vb2 = din("vb2", [2, NC], dt.bfloat16)
    g2rep = din("g2rep", [P, P], dt.float32)
    b2rep = din("b2rep", [P, P], dt.float32)
    iotapw = din("iotapw", [P, 512], dt.bfloat16)
    iotaf4 = din("iotaf4", [P, 512], dt.bfloat16)
    outp = nc.dram_tensor("out", [NQ, P], dt.float32, kind="ExternalOutput").ap()

    with tile.TileContext(nc) as tc, ExitStack() as ctx:
        big = ctx.enter_context(tc.tile_pool(name="big", bufs=1))
        gpool = ctx.enter_context(tc.tile_pool(name="g", bufs=3))
        rvpool = ctx.enter_context(tc.tile_pool(name="rvbc", bufs=3))
        selpool = ctx.enter_context(tc.tile_pool(name="sel", bufs=3))
        mpool = ctx.enter_context(tc.tile_pool(name="msgs", bufs=3))
        sqpool = ctx.enter_context(tc.tile_pool(name="sq", bufs=2))
        smpool = ctx.enter_context(tc.tile_pool(name="sm", bufs=6))
        outpool = ctx.enter_context(tc.tile_pool(name="outs", bufs=3))
        pm = ctx.enter_context(tc.tile_pool(name="pm", bufs=3, space="PSUM"))
        pw = ctx.enter_context(tc.tile_pool(name="pw", bufs=2, space="PSUM"))
        p2 = ctx.enter_context(tc.tile_pool(name="p2", bufs=2, space="PSUM"))

        def load(name, src, shape, dd):
            t = big.tile(shape, dd, tag=name)
            nc.sync.dma_start(t[:], src[:])
            return t

        rvp_sb = load("rvp", rvp, [P, NT], dt.bfloat16)
        nqT_sb = load("nqT", nqT, [P, NQ], dt.bfloat16)
        degones_sb = load("degones", degones, [2, NQ], dt.bfloat16)
        w1s_sb = load("w1s", w1s, [P, NC], dt.bfloat16)
        w1r_sb = load("w1r", w1r, [P, NC], dt.bfloat16)
        baug_sb = load("baug", baug, [P, NC], dt.bfloat16)
        wntop_sb = load("wntop", wntop, [P, NC], dt.bfloat16)
        wnbot_sb = load("wnbot", wnbot, [P, NC], dt.bfloat16)
        vb2_sb = load("vb2", vb2, [2, NC], dt.bfloat16)
        g2rep_sb = load("g2rep", g2rep, [P, P], dt.float32)
        b2rep_sb = load("b2rep", b2rep, [P, P], dt.float32)
        iotapw_sb = load("iotapw", iotapw, [P, 512], dt.bfloat16)
        iotaf4_sb = load("iotaf4", iotaf4, [P, 512], dt.bfloat16)
        y_sb = big.tile([P, NW * NC], dt.bfloat16, tag="y")
        inbox = big.tile([P, NQ], dt.bfloat16, tag="inbox")
        eps_sb = big.tile([P, 1], dt.float32, tag="eps")
        nc.vector.memset(eps_sb[:], float(EPS))

        # ---- phase 0.5: y = nodes_q @ W1r_aug + b_aug
        for w in range(NW):
            yp = p2.tile([P, NC], dt.float32, tag="p2")
            nc.tensor.matmul(out=yp[:], lhsT=nqT_sb[:, w * P:(w + 1) * P],
                             rhs=w1r_sb[:], start=True, stop=True)
            nc.vector.tensor_tensor(
                out=y_sb[:, w * NC:(w + 1) * NC], in0=yp[:],
                in1=baug_sb[:], op=OP.add)

        # ---- main tile loop
        gt = None
        gt_base = 0
        psw = None
        sel4 = selT4 = None
        pair_state = {}
        for t in range(NT):
            w = int(tiles[t])
            if t % CH_TILES == 0:
                ntile = min(CH_TILES, NT - t)
                gt = gpool.tile([P, CH_TILES * P], dt.bfloat16, tag="gt")
                nc.sync.dma_start(gt[:, 0:ntile * P],
                                  sgathT[:, t * P:(t + ntile) * P])
                gt_base = t
            if t % 4 == 0:
                g4 = t // 4
                rvbc = rvpool.tile([P, 512], dt.bfloat16, tag="rvbc")
                nc.sync.dma_start(rvbc[:], rvf[g4:g4 + 1, :].to_broadcast((P, 512)))
                sel4 = selpool.tile([P, 4, P], dt.bfloat16, tag="sel4")
                nc.vector.tensor_tensor(
                    out=sel4[:],
                    in0=rvp_sb[:, t:t + 4].to_broadcast([P, 4, P]),
                    in1=iotaf4_sb[:].rearrange("p (a b) -> p a b", b=P),
                    op=OP.is_equal)
                selT4 = selpool.tile([P, 4, P], dt.bfloat16, tag="selT4")
                nc.vector.tensor_tensor(
                    out=selT4[:],
                    in0=iotapw_sb[:].rearrange("p (a b) -> p a b", b=P),
                    in1=rvbc[:].rearrange("p (a b) -> p a b", b=P),
                    op=OP.is_equal)

            toff = t - gt_base
            j = t % 2
            if j == 0:
                psm2 = pm.tile([P, 2 * NC], dt.float32, tag="pm")
                pair_state = {"psm2": psm2, "tiles": []}
            psm2 = pair_state["psm2"]
            base = j * NC
            nc.tensor.matmul(out=psm2[:, base:base + NC],
                             lhsT=gt[:, toff * P:(toff + 1) * P],
                             rhs=w1s_sb[:], start=True, stop=False)
            nc.tensor.matmul(out=psm2[:, base:base + NC],
                             lhsT=selT4[:, t % 4, :],
                             rhs=y_sb[:, w * NC:(w + 1) * NC],
                             start=False, stop=True)
            pair_state["tiles"].append((t, w, sel4))
            if j == 0:
                continue
            # stats for the pair in batched [P, 2] ops
            negmu2t = smpool.tile([P, 2], dt.float32, tag="negmu")
            nc.vector.tensor_copy(
                out=negmu2t[:],
                in_=psm2[:].rearrange("p (a b) -> p a b", b=NC)[:, :, P])
            ssq2t = smpool.tile([P, 2], dt.float32, tag="ssq")
            sq = sqpool.tile([P, P], dt.bfloat16, tag="sq")
            nc.scalar.activation(sq[:], psm2[:, 0:P], AF.Square,
                                 bias=negmu2t[:, 0:1], scale=1.0,
                                 accum_out=ssq2t[:, 0:1])
            sqb = sqpool.tile([P, P], dt.bfloat16, tag="sqb")
            nc.scalar.activation(sqb[:], psm2[:, NC:NC + P], AF.Square,
                                 bias=negmu2t[:, 1:2], scale=1.0,
                                 accum_out=ssq2t[:, 1:2])
            std2t = smpool.tile([P, 2], dt.float32, tag="std")
            nc.scalar.activation(std2t[:], ssq2t[:], AF.Sqrt,
                                 bias=eps_sb[:], scale=1.0 / P)
            rstd2t = smpool.tile([P, 2], dt.float32, tag="rstd")
            nc.vector.reciprocal(rstd2t[:], std2t[:])
            nmr2t = smpool.tile([P, 2], dt.float32, tag="nmr")
            nc.vector.tensor_mul(out=nmr2t[:], in0=negmu2t[:], in1=rstd2t[:])
            for (tt, ww, sel4t) in pair_state["tiles"]:
                jj = tt % 2
                bb = jj * NC
                msgs = mpool.tile([P, P], dt.bfloat16, tag="msgs")
                if jj == 0:
                    nc.vector.tensor_scalar(
                        out=msgs[:], in0=psm2[:, bb:bb + P],
                        scalar1=negmu2t[:, jj:jj + 1],
                        scalar2=rstd2t[:, jj:jj + 1], op0=OP.add, op1=OP.mult)
                else:
                    nc.scalar.activation(msgs[:], psm2[:, bb:bb + P],
                                         AF.Identity,
                                         bias=nmr2t[:, jj:jj + 1],
                                         scale=rstd2t[:, jj:jj + 1])
                first = tt == cell_off[ww]
                last = tt == cell_off[ww + 1] - 1
                if first:
                    psw = pw.tile([P, P], dt.float32, tag="pw")
                nc.tensor.matmul(out=psw[:], lhsT=msgs[:],
                                 rhs=sel4t[:, tt % 4, :],
                                 start=first, stop=last)
                if last:
                    nc.vector.tensor_copy(out=inbox[:, ww * P:(ww + 1) * P],
                                          in_=psw[:])

        # ---- phase 2
        for w in range(NW):
            ps = p2.tile([P, NC], dt.float32, tag="p2")
            sl = slice(w * P, (w + 1) * P)
            nc.tensor.matmul(out=ps[:], lhsT=degones_sb[:, sl], rhs=vb2_sb[:],
                             start=True, stop=False)
            nc.tensor.matmul(out=ps[:], lhsT=nqT_sb[:, sl], rhs=wntop_sb[:],
                             start=False, stop=False)
            nc.tensor.matmul(out=ps[:], lhsT=inbox[:, sl], rhs=wnbot_sb[:],
                             start=False, stop=True)
            negmu2 = smpool.tile([P, 1], dt.float32, tag="negmu2")
            nc.vector.tensor_copy(out=negmu2[:], in_=ps[:, P:P + 1])
            sq2 = sqpool.tile([P, P], dt.bfloat16, tag="sq2")
            ssq2 = smpool.tile([P, 1], dt.float32, tag="ssq2")
            nc.scalar.activation(sq2[:], ps[:, 0:P], AF.Square,
                                 bias=negmu2[:], scale=1.0, accum_out=ssq2[:])
            std2 = smpool.tile([P, 1], dt.float32, tag="std2")
            nc.scalar.activation(std2[:], ssq2[:], AF.Sqrt,
                                 bias=eps_sb[:], scale=1.0 / P)
            rstd2 = smpool.tile([P, 1], dt.float32, tag="rstd2")
            nc.vector.reciprocal(rstd2[:], std2[:])
            tn = outpool.tile([P, P], dt.float32, tag="tn")
            nc.vector.tensor_scalar(out=tn[:], in0=ps[:, 0:P], scalar1=negmu2[:],
                                    scalar2=rstd2[:], op0=OP.add, op1=OP.mult)
            t2 = outpool.tile([P, P], dt.float32, tag="t2")
            nc.vector.tensor_tensor(out=t2[:], in0=tn[:], in1=g2rep_sb[:],
                                    op=OP.mult)
            osb = outpool.tile([P, P], dt.float32, tag="osb")
            nc.vector.tensor_tensor(out=osb[:], in0=t2[:], in1=b2rep_sb[:],
                                    op=OP.add)
            nc.sync.dma_start(outp[sl, :], osb[:])

    nc.compile()
    return nc


# ----------------------------------------------------------------------------
# entry point
# ----------------------------------------------------------------------------

def kernel(nodes, senders, receivers, W_msg, b_msg, W_node, b_node,
           g1, be1, g2, be2):
    global LAST_EXEC_NS, LAST_RESULTS
    from concourse.bass_utils import run_bass_kernel_spmd

    nodes = np.asarray(nodes, np.float32)
    sched, in_maps, meta = _prep(
        nodes, np.asarray(senders), np.asarray(receivers),
        np.asarray(W_msg, np.float32), np.asarray(b_msg, np.float32),
        np.asarray(W_node, np.float32), np.asarray(b_node, np.float32),
        np.asarray(g1, np.float32), np.asarray(be1, np.float32),
        np.asarray(g2, np.float32), np.asarray(be2, np.float32))
    nc = _build(sched, meta)
    res = run_bass_kernel_spmd(nc, in_maps, list(range(NCORES)), trace=_TRACE)
    LAST_EXEC_NS = res.exec_time_ns
    LAST_RESULTS = res
    B, N, Q, NQR = meta["B"], meta["N"], meta["Q"], meta["NQR"]
    out = np.zeros((B, N, P), np.float32)
    for c in range(NCORES):
        b, q = c // Q, c % Q
        r0 = q * NQR
        r1 = min(r0 + NQR, N)
        out[b, r0:r1, :] = res.results[c]["out"][:r1 - r0, :]
    return out


# revision 21
# speedup vs baseline: 1.3626x; 1.0790x over previous
"""GNN message-passing block on 8 Trainium2 NeuronCores.

Full (unsharded) numpy inputs in, full output out.

Sharding: batch dim across core groups (B=2 -> 4 cores per batch); within a
batch, edges are partitioned by receiver quarter (the scatter_add target
range), so each core owns a disjoint receiver range and no cross-core
communication is needed. Host-side shard construction sorts each core's
edges by receiver window (128 receivers), pads each window to a multiple of
128 edges, and lays out the sender node features in schedule order
(feature-major bf16) so the device streams them sequentially as matmul lhsT.

Per-core device program (bf16 matmuls, f32 PSUM):
  phase 0.5: y[n,:] = nodes_q[n,:] @ W_msg[128:,:] + b_msg  (col 128 = -mean)
  per 128-edge tile:
    psum[e,0:132] = S_T.T @ W1s_aug + SelT.T @ y_win   (col 128 = -mean(msg))
      Sel/SelT are one-hot receiver matrices built on DVE via is_equal
      against iota constants (SelT input row replicated via DMA broadcast).
    LN: ACT square+accum -> var; ACT sqrt; DVE reciprocal; normalize -> bf16
    scatter: psum_w[f,0:128] += msgs.T @ Sel   (segment-sum by receiver)
  per window: flush psum_w -> inbox (bf16, feature-major [128f, NQ])
  phase 2: out_pre = nodes_q@Wn_top + inbox@(g1*Wn_bot) + deg*(be1@Wn_bot)
           + b_node (rank-2 matmul), then LN2 with g2/be2 -> f32 out.
"""

import os
import numpy as np
import ml_dtypes

BF16 = ml_dtypes.bfloat16
P = 128
NC = 132          # matmul N: 128 features + col 128 = -mean + 3 pad
CH_TILES = 32     # tiles per sender-feature load chunk (32*128 edges = 1MB)
EPS = 1e-5
NCORES = 8

# set by test harness for profiling
_TRACE = False
LAST_EXEC_NS = None
LAST_RESULTS = None


# ----------------------------------------------------------------------------
# host-side schedule + per-core tensor prep
# ----------------------------------------------------------------------------

def _dims(nodes):
    B, N, D = nodes.shape
    assert D == P
    Q = NCORES // B
    NQR = -(-N // Q)              # receivers per quarter (real)
    NW = -(-NQR // P)             # windows per quarter
    NQ = NW * P
    return B, N, Q, NQR, NW, NQ


def _make_schedule(counts, NW):
    T = np.maximum(-(-counts.max(axis=0) // P), 1)   # tiles per window
    NT = int(T.sum())
    pad = (-NT) % 4
    if pad:
        T[NW - 1] += pad
        NT += pad
    cell_off = np.zeros(NW + 1, np.int64)
    cell_off[1:] = np.cumsum(T)
    tiles = np.repeat(np.arange(NW), T)              # tile -> window
    return dict(T=T, NT=NT, cell_off=cell_off, tiles=tiles, NW=NW)


def _aug(Wcols):
    K = Wcols.shape[0]
    out = np.zeros((K, NC), np.float32)
    out[:, :P] = Wcols
    out[:, P] = -Wcols.mean(axis=1)
    return out


def _prep(nodes, senders, receivers, W_msg, b_msg, W_node, b_node,
          g1, be1, g2, be2):
    B, N, Q, NQR, NW, NQ = _dims(nodes)

    W1s = W_msg[:P, :].astype(np.float32)
    W1r = W_msg[P:, :].astype(np.float32)
    Wn_top = W_node[:P, :].astype(np.float32)
    Wn_bot = W_node[P:, :].astype(np.float32)
    WnbotF = (g1[:, None] * Wn_bot).astype(np.float32)
    v = (be1 @ Wn_bot).astype(np.float32)
    w1s_aug = _aug(W1s).astype(BF16)
    w1r_aug = _aug(W1r).astype(BF16)
    baug = np.zeros((1, NC), np.float32)
    baug[0, :P] = b_msg
    baug[0, P] = -b_msg.mean()
    baug = np.tile(baug, (P, 1)).astype(BF16)
    wntop_aug = _aug(Wn_top).astype(BF16)
    wnbot_aug = _aug(WnbotF).astype(BF16)
    vb2 = np.zeros((2, NC), np.float32)
    vb2[0, :P] = v
    vb2[0, P] = -v.mean()
    vb2[1, :P] = b_node
    vb2[1, P] = -b_node.mean()
    vb2 = vb2.astype(BF16)
    g2rep = np.tile(g2[None, :], (P, 1)).astype(np.float32)
    b2rep = np.tile(be2[None, :], (P, 1)).astype(np.float32)
    iotapw = np.tile(np.arange(P, dtype=np.float32)[:, None], (1, 512)).astype(BF16)
    iotaf4 = np.tile(np.tile(np.arange(P, dtype=np.float32)[None, :], (1, 4)),
                     (P, 1)).astype(BF16)

    core_edges = []
    counts = np.zeros((NCORES, NW), np.int64)
    for c in range(NCORES):
        b, q = c // Q, c % Q
        r0 = q * NQR
        r1 = min(r0 + NQR, N)
        m = (receivers[b] >= r0) & (receivers[b] < r1)
        s = senders[b][m].astype(np.int64)
        r = (receivers[b][m] - r0).astype(np.int64)
        w = r >> 7
        counts[c] = np.bincount(w, minlength=NW)
        core_edges.append((b, q, s, r, w))
    sched = _make_schedule(counts, NW)
    NT = sched["NT"]
    cell_off = sched["cell_off"]

    in_maps = []
    nodes_bf_cache = {}
    for c in range(NCORES):
        b, q, s, r, w = core_edges[c]
        if b not in nodes_bf_cache:
            nodes_bf_cache[b] = nodes[b].astype(BF16)
        order = np.argsort(w, kind="stable")
        ws = w[order]
        starts = np.searchsorted(ws, np.arange(NW))
        ranks = np.arange(len(order)) - starts[ws]
        slots = cell_off[ws] * P + ranks
        rv_arr = np.full(NT * P, 200.0, np.float32)
        rv_arr[slots] = (r[order] & 127).astype(np.float32)
        sgathT = np.zeros((P, NT * P), BF16)
        sgathT[:, slots] = nodes_bf_cache[b][s[order]].T
        rvp = np.ascontiguousarray(rv_arr.reshape(NT, P).T).astype(BF16)
        rvf = np.ascontiguousarray(rv_arr.reshape(NT // 4, 512)).astype(BF16)
        r0 = q * NQR
        nqr_c = min(NQR, N - r0)
        nqT = np.zeros((P, NQ), BF16)
        nqT[:, :nqr_c] = nodes[b, r0:r0 + nqr_c, :].T.astype(BF16)
        deg = np.bincount(r, minlength=NQ).astype(np.float32)
        degones = np.stack([deg, np.ones(NQ, np.float32)]).astype(BF16)
        in_maps.append({
            "sgathT": sgathT,
            "nqT": nqT,
            "degones": degones,
            "rvp": rvp,
            "rvf": rvf,
            "w1s": w1s_aug, "w1r": w1r_aug, "baug": baug,
            "wntop": wntop_aug, "wnbot": wnbot_aug, "vb2": vb2,
            "g2rep": g2rep, "b2rep": b2rep, "iotapw": iotapw, "iotaf4": iotaf4,
        })
    meta = dict(B=B, N=N, Q=Q, NQR=NQR, NW=NW, NQ=NQ)
    return sched, in_maps, meta


# ----------------------------------------------------------------------------
# device program
# ----------------------------------------------------------------------------

def _build(sched, meta):
    import concourse.bacc as bacc
    import concourse.tile as tile
    from concourse import mybir
    from contextlib import ExitStack

    dt = mybir.dt
    AF = mybir.ActivationFunctionType
    OP = mybir.AluOpType

    NW, NQ = meta["NW"], meta["NQ"]
    NT = sched["NT"]
    tiles = sched["tiles"]
    cell_off = sched["cell_off"]

    nc = bacc.Bacc("TRN2", target_bir_lowering=False, debug=False,
                   enable_asserts=True, num_devices=NCORES)

    def din(name, shape, dd):
        return nc.dram_tensor(name, shape, dd, kind="ExternalInput").ap()

    sgathT = din("sgathT", [P, NT * P], dt.bfloat16)
    nqT = din("nqT", [P, NQ], dt.bfloat16)
    degones = din("degones", [2, NQ], dt.bfloat16)
    rvp = din("rvp", [P, NT], dt.bfloat16)
    rvf = din("rvf", [NT // 4, 512], dt.bfloat16)
    w1s = din("w1s", [P, NC], dt.bfloat16)
    w1r = din("w1r", [P, NC], dt.bfloat16)
    baug = din("baug", [P, NC], dt.bfloat16)
    wntop = din("wntop", [P, NC], dt.bfloat16)
    wnbot = din("wnbot", [P, NC], dt.bfloat16)
    vb2 = din("vb2", [2, NC], dt.bfloat16)
    g2rep = din("g2rep", [P, P], dt.float32)
    b2rep = din("b2rep", [P, P], dt.float32)
    iotapw = din("iotapw", [P, 512], dt.bfloat16)
    iotaf4 = din("iotaf4", [P, 512], dt.bfloat16)
    outp = nc.dram_tensor("out", [NQ, P], dt.float32, kind="ExternalOutput").ap()

    with tile.TileContext(nc) as tc, ExitStack() as ctx:
        big = ctx.enter_context(tc.tile_pool(name="big", bufs=1))
        gpool = ctx.enter_context(tc.tile_pool(name="g", bufs=3))
        rvpool = ctx.enter_context(tc.tile_pool(name="rvbc", bufs=3))
        selpool = ctx.enter_context(tc.tile_pool(name="sel", bufs=3))
        mpool = ctx.enter_context(tc.tile_pool(name="msgs", bufs=3))
        sqpool = ctx.enter_context(tc.tile_pool(name="sq", bufs=2))
        smpool = ctx.enter_context(tc.tile_pool(name="sm", bufs=6))
        outpool = ctx.enter_context(tc.tile_pool(name="outs", bufs=3))
        pm = ctx.enter_context(tc.tile_pool(name="pm", bufs=4, space="PSUM"))
        pw = ctx.enter_context(tc.tile_pool(name="pw", bufs=2, space="PSUM"))
        p2 = ctx.enter_context(tc.tile_pool(name="p2", bufs=2, space="PSUM"))

        def load(name, src, shape, dd):
            t = big.tile(shape, dd, tag=name)
            nc.sync.dma_start(t[:], src[:])
            return t

        rvp_sb = load("rvp", rvp, [P, NT], dt.bfloat16)
        nqT_sb = load("nqT", nqT, [P, NQ], dt.bfloat16)
        degones_sb = load("degones", degones, [2, NQ], dt.bfloat16)
        w1s_sb = load("w1s", w1s, [P, NC], dt.bfloat16)
        w1r_sb = load("w1r", w1r, [P, NC], dt.bfloat16)
        baug_sb = load("baug", baug, [P, NC], dt.bfloat16)
        wntop_sb = load("wntop", wntop, [P, NC], dt.bfloat16)
        wnbot_sb = load("wnbot", wnbot, [P, NC], dt.bfloat16)
        vb2_sb = load("vb2", vb2, [2, NC], dt.bfloat16)
        g2rep_sb = load("g2rep", g2rep, [P, P], dt.float32)
        b2rep_sb = load("b2rep", b2rep, [P, P], dt.float32)
        iotapw_sb = load("iotapw", iotapw, [P, 512], dt.bfloat16)
        iotaf4_sb = load("iotaf4", iotaf4, [P, 512], dt.bfloat16)
        y_sb = big.tile([P, NW * NC], dt.bfloat16, tag="y")
        inbox = big.tile([P, NQ], dt.bfloat16, tag="inbox")
        eps_sb = big.tile([P, 1], dt.float32, tag="eps")
        nc.vector.memset(eps_sb[:], float(EPS))

        # ---- phase 0.5: y = nodes_q @ W1r_aug + b_aug
        for w in range(NW):
            yp = p2.tile([P, NC], dt.float32, tag="p2")
            nc.tensor.matmul(out=yp[:], lhsT=nqT_sb[:, w * P:(w + 1) * P],
                             rhs=w1r_sb[:], start=True, stop=True)
            nc.vector.tensor_tensor(
                out=y_sb[:, w * NC:(w + 1) * NC], in0=yp[:],
                in1=baug_sb[:], op=OP.add)

        # ---- main tile loop
        gt = None
        gt_base = 0
        psw = None
        sel4 = selT4 = None
        quad = {}
        for t in range(NT):
            w = int(tiles[t])
            if t % CH_TILES == 0:
                ntile = min(CH_TILES, NT - t)
                gt = gpool.tile([P, CH_TILES * P], dt.bfloat16, tag="gt")
                nc.sync.dma_start(gt[:, 0:ntile * P],
                                  sgathT[:, t * P:(t + ntile) * P])
                gt_base = t
            if t % 4 == 0:
                g4 = t // 4
                rvbc = rvpool.tile([P, 512], dt.bfloat16, tag="rvbc")
                nc.sync.dma_start(rvbc[:], rvf[g4:g4 + 1, :].to_broadcast((P, 512)))
                sel4 = selpool.tile([P, 4, P], dt.bfloat16, tag="sel4")
                nc.vector.tensor_tensor(
                    out=sel4[:],
                    in0=rvp_sb[:, t:t + 4].to_broadcast([P, 4, P]),
                    in1=iotaf4_sb[:].rearrange("p (a b) -> p a b", b=P),
                    op=OP.is_equal)
                selT4 = selpool.tile([P, 4, P], dt.bfloat16, tag="selT4")
                nc.vector.tensor_tensor(
                    out=selT4[:],
                    in0=iotapw_sb[:].rearrange("p (a b) -> p a b", b=P),
                    in1=rvbc[:].rearrange("p (a b) -> p a b", b=P),
                    op=OP.is_equal)
                negmu4 = smpool.tile([P, 4], dt.float32, tag="negmu")
                ssq4 = smpool.tile([P, 4], dt.float32, tag="ssq")
                quad = {"negmu": negmu4, "ssq": ssq4, "tiles": []}

            toff = t - gt_base
            j = t % 2
            if j == 0:
                psm2 = pm.tile([P, 2 * NC], dt.float32, tag="pm")
                quad["psm" + str((t % 4) // 2)] = psm2
            base = j * NC
            nc.tensor.matmul(out=psm2[:, base:base + NC],
                             lhsT=gt[:, toff * P:(toff + 1) * P],
                             rhs=w1s_sb[:], start=True, stop=False)
            nc.tensor.matmul(out=psm2[:, base:base + NC],
                             lhsT=selT4[:, t % 4, :],
                             rhs=y_sb[:, w * NC:(w + 1) * NC],
                             start=False, stop=True)
            quad["tiles"].append((t, w, sel4, psm2))
            if j == 1:
                q2 = ((t % 4) // 2) * 2
                nc.vector.tensor_copy(
                    out=quad["negmu"][:, q2:q2 + 2],
                    in_=psm2[:].rearrange("p (a b) -> p a b", b=NC)[:, :, P])
                sq = sqpool.tile([P, P], dt.bfloat16, tag="sq")
                nc.scalar.activation(sq[:], psm2[:, 0:P], AF.Square,
                                     bias=quad["negmu"][:, q2:q2 + 1], scale=1.0,
                                     accum_out=quad["ssq"][:, q2:q2 + 1])
                sqb = sqpool.tile([P, P], dt.bfloat16, tag="sqb")
                nc.scalar.activation(sqb[:], psm2[:, NC:NC + P], AF.Square,
                                     bias=quad["negmu"][:, q2 + 1:q2 + 2], scale=1.0,
                                     accum_out=quad["ssq"][:, q2 + 1:q2 + 2])
            if t % 4 != 3:
                continue
            # batched rstd for the quad
            std4 = smpool.tile([P, 4], dt.float32, tag="std")
            nc.scalar.activation(std4[:], quad["ssq"][:], AF.Sqrt,
                                 bias=eps_sb[:], scale=1.0 / P)
            rstd4 = smpool.tile([P, 4], dt.float32, tag="rstd")
            nc.vector.reciprocal(rstd4[:], std4[:])
            nmr4 = smpool.tile([P, 4], dt.float32, tag="nmr")
            nc.vector.tensor_mul(out=nmr4[:], in0=quad["negmu"][:], in1=rstd4[:])
            for (tt, ww, sel4t, psm2t) in quad["tiles"]:
                k = tt % 4
                bb = (tt % 2) * NC
                msgs = mpool.tile([P, P], dt.bfloat16, tag="msgs")
                if tt % 2 == 0:
                    nc.vector.tensor_scalar(
                        out=msgs[:], in0=psm2t[:, bb:bb + P],
                        scalar1=quad["negmu"][:, k:k + 1],
                        scalar2=rstd4[:, k:k + 1], op0=OP.add, op1=OP.mult)
                else:
                    nc.scalar.activation(msgs[:], psm2t[:, bb:bb + P],
                                         AF.Identity,
                                         bias=nmr4[:, k:k + 1],
                                         scale=rstd4[:, k:k + 1])
                first = tt == cell_off[ww]
                last = tt == cell_off[ww + 1] - 1
                if first:
                    psw = pw.tile([P, P], dt.float32, tag="pw")
                nc.tensor.matmul(out=psw[:], lhsT=msgs[:],
                                 rhs=sel4t[:, tt % 4, :],
                                 start=first, stop=last)
                if last:
                    nc.vector.tensor_copy(out=inbox[:, ww * P:(ww + 1) * P],
                                          in_=psw[:])

        # ---- phase 2
        for w in range(NW):
            ps = p2.tile([P, NC], dt.float32, tag="p2")
            sl = slice(w * P, (w + 1) * P)
            nc.tensor.matmul(out=ps[:], lhsT=degones_sb[:, sl], rhs=vb2_sb[:],
                             start=True, stop=False)
            nc.tensor.matmul(out=ps[:], lhsT=nqT_sb[:, sl], rhs=wntop_sb[:],
                             start=False, stop=False)
            nc.tensor.matmul(out=ps[:], lhsT=inbox[:, sl], rhs=wnbot_sb[:],
                             start=False, stop=True)
            negmu2 = smpool.tile([P, 1], dt.float32, tag="negmu2")
            nc.vector.tensor_copy(out=negmu2[:], in_=ps[:, P:P + 1])
            sq2 = sqpool.tile([P, P], dt.bfloat16, tag="sq2")
            ssq2 = smpool.tile([P, 1], dt.float32, tag="ssq2")
            nc.scalar.activation(sq2[:], ps[:, 0:P], AF.Square,
                                 bias=negmu2[:], scale=1.0, accum_out=ssq2[:])
            std2 = smpool.tile([P, 1], dt.float32, tag="std2")
            nc.scalar.activation(std2[:], ssq2[:], AF.Sqrt,
                                 bias=eps_sb[:], scale=1.0 / P)
            rstd2 = smpool.tile([P, 1], dt.float32, tag="rstd2")
            nc.vector.reciprocal(rstd2[:], std2[:])
            tn = outpool.tile([P, P], dt.float32, tag="tn")
            nc.vector.tensor_scalar(out=tn[:], in0=ps[:, 0:P], scalar1=negmu2[:],
                                    scalar2=rstd2[:], op0=OP.add, op1=OP.mult)
            t2 = outpool.tile([P, P], dt.float32, tag="t2")
            nc.vector.tensor_tensor(out=t2[:], in0=tn[:], in1=g2rep_sb[:],
                                    op=OP.mult)
            osb = outpool.tile([P, P], dt.float32, tag="osb")
            nc.vector.tensor_tensor(out=osb[:], in0=t2[:], in1=b2rep_sb[:],
                                    op=OP.add)
            nc.sync.dma_start(outp[sl, :], osb[:])

    nc.compile()
    return nc


# ----------------------------------------------------------------------------
# entry point
# ----------------------------------------------------------------------------

def kernel(nodes, senders, receivers, W_msg, b_msg, W_node, b_node,
           g1, be1, g2, be2):
    global LAST_EXEC_NS, LAST_RESULTS
    from concourse.bass_utils import run_bass_kernel_spmd

    nodes = np.asarray(nodes, np.float32)
    sched, in_maps, meta = _prep(
        nodes, np.asarray(senders), np.asarray(receivers),
        np.asarray(W_msg, np.float32), np.asarray(b_msg, np.float32),
        np.asarray(W_node, np.float32), np.asarray(b_node, np.float32),
        np.asarray(g1, np.float32), np.asarray(be1, np.float32),
        np.asarray(g2, np.float32), np.asarray(be2, np.float32))
    nc = _build(sched, meta)
    res = run_bass_kernel_spmd(nc, in_maps, list(range(NCORES)), trace=_TRACE)
    LAST_EXEC_NS = res.exec_time_ns
    LAST_RESULTS = res
    B, N, Q, NQR = meta["B"], meta["N"], meta["Q"], meta["NQR"]
    out = np.zeros((B, N, P), np.float32)
    for c in range(NCORES):
        b, q = c // Q, c % Q
        r0 = q * NQR
        r1 = min(r0 + NQR, N)
        out[b, r0:r1, :] = res.results[c]["out"][:r1 - r0, :]
    return out


# revision 22
# speedup vs baseline: 1.3920x; 1.0216x over previous
"""GNN message-passing block on 8 Trainium2 NeuronCores.

Full (unsharded) numpy inputs in, full output out.

Sharding: batch dim across core groups (B=2 -> 4 cores per batch); within a
batch, edges are partitioned by receiver quarter (the scatter_add target
range), so each core owns a disjoint receiver range and no cross-core
communication is needed. Host-side shard construction sorts each core's
edges by receiver window (128 receivers), pads each window to a multiple of
128 edges, and lays out the sender node features in schedule order
(feature-major bf16) so the device streams them sequentially as matmul lhsT.

Per-core device program (bf16 matmuls, f32 PSUM):
  phase 0.5: y[n,:] = nodes_q[n,:] @ W_msg[128:,:] + b_msg  (col 128 = -mean)
  per 128-edge tile:
    psum[e,0:132] = S_T.T @ W1s_aug + SelT.T @ y_win   (col 128 = -mean(msg))
      Sel/SelT are one-hot receiver matrices built on DVE via is_equal
      against iota constants (SelT input row replicated via DMA broadcast).
    LN: ACT square+accum -> var; ACT sqrt; DVE reciprocal; normalize -> bf16
    scatter: psum_w[f,0:128] += msgs.T @ Sel   (segment-sum by receiver)
  per window: flush psum_w -> inbox (bf16, feature-major [128f, NQ])
  phase 2: out_pre = nodes_q@Wn_top + inbox@(g1*Wn_bot) + deg*(be1@Wn_bot)
           + b_node (rank-2 matmul), then LN2 with g2/be2 -> f32 out.
"""

import os
import numpy as np
import ml_dtypes

BF16 = ml_dtypes.bfloat16
P = 128
NC = 132          # matmul N: 128 features + col 128 = -mean + 3 pad
CH_TILES = 32     # tiles per sender-feature load chunk (32*128 edges = 1MB)
EPS = 1e-5
NCORES = 8

# set by test harness for profiling
_TRACE = False
LAST_EXEC_NS = None
LAST_RESULTS = None


# ----------------------------------------------------------------------------
# host-side schedule + per-core tensor prep
# ----------------------------------------------------------------------------

def _dims(nodes):
    B, N, D = nodes.shape
    assert D == P
    Q = NCORES // B
    NQR = -(-N // Q)              # receivers per quarter (real)
    NW = -(-NQR // P)             # windows per quarter
    NQ = NW * P
    return B, N, Q, NQR, NW, NQ


def _make_schedule(counts, NW):
    T = np.maximum(-(-counts.max(axis=0) // P), 1)   # tiles per window
    NT = int(T.sum())
    pad = (-NT) % 4
    if pad:
        T[NW - 1] += pad
        NT += pad
    cell_off = np.zeros(NW + 1, np.int64)
    cell_off[1:] = np.cumsum(T)
    tiles = np.repeat(np.arange(NW), T)              # tile -> window
    return dict(T=T, NT=NT, cell_off=cell_off, tiles=tiles, NW=NW)


def _aug(Wcols):
    K = Wcols.shape[0]
    out = np.zeros((K, NC), np.float32)
    out[:, :P] = Wcols
    out[:, P] = -Wcols.mean(axis=1)
    return out


def _prep(nodes, senders, receivers, W_msg, b_msg, W_node, b_node,
          g1, be1, g2, be2):
    B, N, Q, NQR, NW, NQ = _dims(nodes)

    W1s = W_msg[:P, :].astype(np.float32)
    W1r = W_msg[P:, :].astype(np.float32)
    Wn_top = W_node[:P, :].astype(np.float32)
    Wn_bot = W_node[P:, :].astype(np.float32)
    WnbotF = (g1[:, None] * Wn_bot).astype(np.float32)
    v = (be1 @ Wn_bot).astype(np.float32)
    w1s_aug = _aug(W1s).astype(BF16)
    w1r_aug = _aug(W1r).astype(BF16)
    baug = np.zeros((1, NC), np.float32)
    baug[0, :P] = b_msg
    baug[0, P] = -b_msg.mean()
    baug = np.tile(baug, (P, 1)).astype(BF16)
    wntop_aug = _aug(Wn_top).astype(BF16)
    wnbot_aug = _aug(WnbotF).astype(BF16)
    vb2 = np.zeros((2, NC), np.float32)
    vb2[0, :P] = v
    vb2[0, P] = -v.mean()
    vb2[1, :P] = b_node
    vb2[1, P] = -b_node.mean()
    vb2 = vb2.astype(BF16)
    g2rep = np.tile(g2[None, :], (P, 1)).astype(np.float32)
    b2rep = np.tile(be2[None, :], (P, 1)).astype(np.float32)
    iotapw = np.tile(np.arange(P, dtype=np.float32)[:, None], (1, 512)).astype(BF16)
    iotaf4 = np.tile(np.tile(np.arange(P, dtype=np.float32)[None, :], (1, 4)),
                     (P, 1)).astype(BF16)

    core_edges = []
    counts = np.zeros((NCORES, NW), np.int64)
    for c in range(NCORES):
        b, q = c // Q, c % Q
        r0 = q * NQR
        r1 = min(r0 + NQR, N)
        m = (receivers[b] >= r0) & (receivers[b] < r1)
        s = senders[b][m].astype(np.int64)
        r = (receivers[b][m] - r0).astype(np.int64)
        w = r >> 7
        counts[c] = np.bincount(w, minlength=NW)
        core_edges.append((b, q, s, r, w))
    sched = _make_schedule(counts, NW)
    NT = sched["NT"]
    cell_off = sched["cell_off"]

    in_maps = []
    nodes_bf_cache = {}
    for c in range(NCORES):
        b, q, s, r, w = core_edges[c]
        if b not in nodes_bf_cache:
            nodes_bf_cache[b] = nodes[b].astype(BF16)
        order = np.argsort(w, kind="stable")
        ws = w[order]
        starts = np.searchsorted(ws, np.arange(NW))
        ranks = np.arange(len(order)) - starts[ws]
        slots = cell_off[ws] * P + ranks
        rv_arr = np.full(NT * P, 200.0, np.float32)
        rv_arr[slots] = (r[order] & 127).astype(np.float32)
        sgathT = np.zeros((P, NT * P), BF16)
        sgathT[:, slots] = nodes_bf_cache[b][s[order]].T
        rvp = np.ascontiguousarray(rv_arr.reshape(NT, P).T).astype(BF16)
        rvf = np.ascontiguousarray(rv_arr.reshape(NT // 4, 512)).astype(BF16)
        r0 = q * NQR
        nqr_c = min(NQR, N - r0)
        nqT = np.zeros((P, NQ), BF16)
        nqT[:, :nqr_c] = nodes[b, r0:r0 + nqr_c, :].T.astype(BF16)
        deg = np.bincount(r, minlength=NQ).astype(np.float32)
        degones = np.stack([deg, np.ones(NQ, np.float32)]).astype(BF16)
        in_maps.append({
            "sgathT": sgathT,
            "nqT": nqT,
            "degones": degones,
            "rvp": rvp,
            "rvf": rvf,
            "w1s": w1s_aug, "w1r": w1r_aug, "baug": baug,
            "wntop": wntop_aug, "wnbot": wnbot_aug, "vb2": vb2,
            "g2rep": g2rep, "b2rep": b2rep, "iotapw": iotapw, "iotaf4": iotaf4,
        })
    meta = dict(B=B, N=N, Q=Q, NQR=NQR, NW=NW, NQ=NQ)
    return sched, in_maps, meta


# ----------------------------------------------------------------------------
# device program
# ----------------------------------------------------------------------------

def _build(sched, meta):
    import concourse.bacc as bacc
    import concourse.tile as tile
    from concourse import mybir
    from contextlib import ExitStack

    dt = mybir.dt
    AF = mybir.ActivationFunctionType
    OP = mybir.AluOpType

    NW, NQ = meta["NW"], meta["NQ"]
    NT = sched["NT"]
    tiles = sched["tiles"]
    cell_off = sched["cell_off"]

    nc = bacc.Bacc("TRN2", target_bir_lowering=False, debug=False,
                   enable_asserts=True, num_devices=NCORES)

    def din(name, shape, dd):
        return nc.dram_tensor(name, shape, dd, kind="ExternalInput").ap()

    sgathT = din("sgathT", [P, NT * P], dt.bfloat16)
    nqT = din("nqT", [P, NQ], dt.bfloat16)
    degones = din("degones", [2, NQ], dt.bfloat16)
    rvp = din("rvp", [P, NT], dt.bfloat16)
    rvf = din("rvf", [NT // 4, 512], dt.bfloat16)
    w1s = din("w1s", [P, NC], dt.bfloat16)
    w1r = din("w1r", [P, NC], dt.bfloat16)
    baug = din("baug", [P, NC], dt.bfloat16)
    wntop = din("wntop", [P, NC], dt.bfloat16)
    wnbot = din("wnbot", [P, NC], dt.bfloat16)
    vb2 = din("vb2", [2, NC], dt.bfloat16)
    g2rep = din("g2rep", [P, P], dt.float32)
    b2rep = din("b2rep", [P, P], dt.float32)
    iotapw = din("iotapw", [P, 512], dt.bfloat16)
    iotaf4 = din("iotaf4", [P, 512], dt.bfloat16)
    outp = nc.dram_tensor("out", [NQ, P], dt.float32, kind="ExternalOutput").ap()

    with tile.TileContext(nc) as tc, ExitStack() as ctx:
        big = ctx.enter_context(tc.tile_pool(name="big", bufs=1))
        gpool = ctx.enter_context(tc.tile_pool(name="g", bufs=3))
        rvpool = ctx.enter_context(tc.tile_pool(name="rvbc", bufs=3))
        selpool = ctx.enter_context(tc.tile_pool(name="sel", bufs=3))
        mpool = ctx.enter_context(tc.tile_pool(name="msgs", bufs=3))
        sqpool = ctx.enter_context(tc.tile_pool(name="sq", bufs=2))
        smpool = ctx.enter_context(tc.tile_pool(name="sm", bufs=6))
        outpool = ctx.enter_context(tc.tile_pool(name="outs", bufs=3))
        pm = ctx.enter_context(tc.tile_pool(name="pm", bufs=4, space="PSUM"))
        pw = ctx.enter_context(tc.tile_pool(name="pw", bufs=2, space="PSUM"))
        p2 = ctx.enter_context(tc.tile_pool(name="p2", bufs=2, space="PSUM"))

        def load(name, src, shape, dd):
            t = big.tile(shape, dd, tag=name)
            nc.sync.dma_start(t[:], src[:])
            return t

        rvp_sb = load("rvp", rvp, [P, NT], dt.bfloat16)
        nqT_sb = load("nqT", nqT, [P, NQ], dt.bfloat16)
        degones_sb = load("degones", degones, [2, NQ], dt.bfloat16)
        w1s_sb = load("w1s", w1s, [P, NC], dt.bfloat16)
        w1r_sb = load("w1r", w1r, [P, NC], dt.bfloat16)
        baug_sb = load("baug", baug, [P, NC], dt.bfloat16)
        wntop_sb = load("wntop", wntop, [P, NC], dt.bfloat16)
        wnbot_sb = load("wnbot", wnbot, [P, NC], dt.bfloat16)
        vb2_sb = load("vb2", vb2, [2, NC], dt.bfloat16)
        g2rep_sb = load("g2rep", g2rep, [P, P], dt.float32)
        b2rep_sb = load("b2rep", b2rep, [P, P], dt.float32)
        iotapw_sb = load("iotapw", iotapw, [P, 512], dt.bfloat16)
        iotaf4_sb = load("iotaf4", iotaf4, [P, 512], dt.bfloat16)
        y_sb = big.tile([P, NW * NC], dt.bfloat16, tag="y")
        inbox = big.tile([P, NQ], dt.bfloat16, tag="inbox")
        eps_sb = big.tile([P, 1], dt.float32, tag="eps")
        nc.vector.memset(eps_sb[:], float(EPS))

        # ---- phase 0.5: y = nodes_q @ W1r_aug + b_aug
        for w in range(NW):
            yp = p2.tile([P, NC], dt.float32, tag="p2")
            nc.tensor.matmul(out=yp[:], lhsT=nqT_sb[:, w * P:(w + 1) * P],
                             rhs=w1r_sb[:], start=True, stop=True)
            nc.vector.tensor_tensor(
                out=y_sb[:, w * NC:(w + 1) * NC], in0=yp[:],
                in1=baug_sb[:], op=OP.add)

        # ---- main tile loop
        gt = None
        gt_base = 0
        psw = None
        sel4 = selT4 = None
        quad = {}
        for t in range(NT):
            w = int(tiles[t])
            if t % CH_TILES == 0:
                ntile = min(CH_TILES, NT - t)
                gt = gpool.tile([P, CH_TILES * P], dt.bfloat16, tag="gt")
                nc.sync.dma_start(gt[:, 0:ntile * P],
                                  sgathT[:, t * P:(t + ntile) * P])
                gt_base = t
            if t % 4 == 0:
                g4 = t // 4
                rvbc = rvpool.tile([P, 512], dt.bfloat16, tag="rvbc")
                nc.sync.dma_start(rvbc[:], rvf[g4:g4 + 1, :].to_broadcast((P, 512)))
                sel4 = selpool.tile([P, 4, P], dt.bfloat16, tag="sel4")
                nc.vector.tensor_tensor(
                    out=sel4[:],
                    in0=rvp_sb[:, t:t + 4].to_broadcast([P, 4, P]),
                    in1=iotaf4_sb[:].rearrange("p (a b) -> p a b", b=P),
                    op=OP.is_equal)
                selT4 = selpool.tile([P, 4, P], dt.bfloat16, tag="selT4")
                nc.vector.tensor_tensor(
                    out=selT4[:],
                    in0=iotapw_sb[:].rearrange("p (a b) -> p a b", b=P),
                    in1=rvbc[:].rearrange("p (a b) -> p a b", b=P),
                    op=OP.is_equal)
                negmu4 = smpool.tile([P, 4], dt.float32, tag="negmu")
                ssq4 = smpool.tile([P, 4], dt.float32, tag="ssq")
                quad = {"negmu": negmu4, "ssq": ssq4, "tiles": []}

            toff = t - gt_base
            j = t % 2
            if j == 0:
                psm2 = pm.tile([P, 2 * NC], dt.float32, tag="pm")
                quad["psm" + str((t % 4) // 2)] = psm2
            base = j * NC
            nc.tensor.matmul(out=psm2[:, base:base + NC],
                             lhsT=gt[:, toff * P:(toff + 1) * P],
                             rhs=w1s_sb[:], start=True, stop=False)
            nc.tensor.matmul(out=psm2[:, base:base + NC],
                             lhsT=selT4[:, t % 4, :],
                             rhs=y_sb[:, w * NC:(w + 1) * NC],
                             start=False, stop=True)
            quad["tiles"].append((t, w, sel4, psm2))
            if j == 1:
                q2 = ((t % 4) // 2) * 2
                nc.vector.tensor_copy(
                    out=quad["negmu"][:, q2:q2 + 2],
                    in_=psm2[:].rearrange("p (a b) -> p a b", b=NC)[:, :, P])
                sq = sqpool.tile([P, P], dt.bfloat16, tag="sq")
                nc.scalar.activation(sq[:], psm2[:, 0:P], AF.Square,
                                     bias=quad["negmu"][:, q2:q2 + 1], scale=1.0,
                                     accum_out=quad["ssq"][:, q2:q2 + 1])
                sqb = sqpool.tile([P, P], dt.bfloat16, tag="sqb")
                nc.scalar.activation(sqb[:], psm2[:, NC:NC + P], AF.Square,
                                     bias=quad["negmu"][:, q2 + 1:q2 + 2], scale=1.0,
                                     accum_out=quad["ssq"][:, q2 + 1:q2 + 2])
            if t % 4 != 3:
                continue
            # batched rstd for the quad
            std4 = smpool.tile([P, 4], dt.float32, tag="std")
            nc.scalar.activation(std4[:], quad["ssq"][:], AF.Sqrt,
                                 bias=eps_sb[:], scale=1.0 / P)
            rstd4 = smpool.tile([P, 4], dt.float32, tag="rstd")
            nc.vector.reciprocal(rstd4[:], std4[:])
            for (tt, ww, sel4t, psm2t) in quad["tiles"]:
                k = tt % 4
                bb = (tt % 2) * NC
                msgs = mpool.tile([P, P], dt.bfloat16, tag="msgs")
                nc.vector.tensor_scalar(
                    out=msgs[:], in0=psm2t[:, bb:bb + P],
                    scalar1=quad["negmu"][:, k:k + 1],
                    scalar2=rstd4[:, k:k + 1], op0=OP.add, op1=OP.mult)
                first = tt == cell_off[ww]
                last = tt == cell_off[ww + 1] - 1
                if first:
                    psw = pw.tile([P, P], dt.float32, tag="pw")
                nc.tensor.matmul(out=psw[:], lhsT=msgs[:],
                                 rhs=sel4t[:, tt % 4, :],
                                 start=first, stop=last)
                if last:
                    nc.vector.tensor_copy(out=inbox[:, ww * P:(ww + 1) * P],
                                          in_=psw[:])

        # ---- phase 2
        for w in range(NW):
            ps = p2.tile([P, NC], dt.float32, tag="p2")
            sl = slice(w * P, (w + 1) * P)
            nc.tensor.matmul(out=ps[:], lhsT=degones_sb[:, sl], rhs=vb2_sb[:],
                             start=True, stop=False)
            nc.tensor.matmul(out=ps[:], lhsT=nqT_sb[:, sl], rhs=wntop_sb[:],
                             start=False, stop=False)
            nc.tensor.matmul(out=ps[:], lhsT=inbox[:, sl], rhs=wnbot_sb[:],
                             start=False, stop=True)
            negmu2 = smpool.tile([P, 1], dt.float32, tag="negmu2")
            nc.vector.tensor_copy(out=negmu2[:], in_=ps[:, P:P + 1])
            sq2 = sqpool.tile([P, P], dt.bfloat16, tag="sq2")
            ssq2 = smpool.tile([P, 1], dt.float32, tag="ssq2")
            nc.scalar.activation(sq2[:], ps[:, 0:P], AF.Square,
                                 bias=negmu2[:], scale=1.0, accum_out=ssq2[:])
            std2 = smpool.tile([P, 1], dt.float32, tag="std2")
            nc.scalar.activation(std2[:], ssq2[:], AF.Sqrt,
                                 bias=eps_sb[:], scale=1.0 / P)
            rstd2 = smpool.tile([P, 1], dt.float32, tag="rstd2")
            nc.vector.reciprocal(rstd2[:], std2[:])
            tn = outpool.tile([P, P], dt.float32, tag="tn")
            nc.vector.tensor_scalar(out=tn[:], in0=ps[:, 0:P], scalar1=negmu2[:],
                                    scalar2=rstd2[:], op0=OP.add, op1=OP.mult)
            t2 = outpool.tile([P, P], dt.float32, tag="t2")
            nc.vector.tensor_tensor(out=t2[:], in0=tn[:], in1=g2rep_sb[:],
                                    op=OP.mult)
            osb = outpool.tile([P, P], dt.float32, tag="osb")
            nc.vector.tensor_tensor(out=osb[:], in0=t2[:], in1=b2rep_sb[:],
                                    op=OP.add)
            nc.sync.dma_start(outp[sl, :], osb[:])

    nc.compile()
    return nc


# ----------------------------------------------------------------------------
# entry point
# ----------------------------------------------------------------------------

def kernel(nodes, senders, receivers, W_msg, b_msg, W_node, b_node,
           g1, be1, g2, be2):
    global LAST_EXEC_NS, LAST_RESULTS
    from concourse.bass_utils import run_bass_kernel_spmd

    nodes = np.asarray(nodes, np.float32)
    sched, in_maps, meta = _prep(
        nodes, np.asarray(senders), np.asarray(receivers),
        np.asarray(W_msg, np.float32), np.asarray(b_msg, np.float32),
        np.asarray(W_node, np.float32), np.asarray(b_node, np.float32),
        np.asarray(g1, np.float32), np.asarray(be1, np.float32),
        np.asarray(g2, np.float32), np.asarray(be2, np.float32))
    nc = _build(sched, meta)
    res = run_bass_kernel_spmd(nc, in_maps, list(range(NCORES)), trace=_TRACE)
    LAST_EXEC_NS = res.exec_time_ns
    LAST_RESULTS = res
    B, N, Q, NQR = meta["B"], meta["N"], meta["Q"], meta["NQR"]
    out = np.zeros((B, N, P), np.float32)
    for c in range(NCORES):
        b, q = c // Q, c % Q
        r0 = q * NQR
        r1 = min(r0 + NQR, N)
        out[b, r0:r1, :] = res.results[c]["out"][:r1 - r0, :]
    return out
